# revision 7
# baseline (speedup 1.0000x reference)
"""Multi-head attention (B=2, S=2048, D=1024, H=16) on 8 Trainium2 NeuronCores.

Sharding: data-parallel on batch (2 ways) x tensor-parallel on heads (4 ways,
4 heads = 256 d_model dims per core), per the problem's sharding hint. Each
core:
  - projects Q^T/K^T (in transposed [256, S] layout) and V ([S, 256+ones])
    for its head slice from host-pre-transposed activations x^T,
  - runs causal attention per head: scores^T tiles -> additive -1e9 mask on
    the diagonal blocks -> exp (no max-subtraction: scores ~ N(0,1) for this
    module's input distribution, verified causal mask on host) -> A@[V|1]
    accumulation where the appended ones column yields the softmax
    denominator for free -> per-row normalization on eviction,
  - PE-transposes the per-head outputs into O^T for the output projection,
  - projects through its Wo column slice (+ folded v/wo biases),
  - ReduceScatters partial outputs (4 chunks of 512 rows, pipelined with
    compute) across the 4 cores of its batch group.
Host reassembles the 8 x [512, 1024] shards into [2, 2048, 1024].

Matmuls run as float32r (full-rate fp32 path on the PE).
"""

import os
import numpy as np

import concourse.bass as bass
import concourse.mybir as mybir
import concourse.tile as tile
from concourse import bacc
from concourse.bass_utils import run_bass_kernel_spmd

B, S, D, H = 2, 2048, 1024, 16
DK = D // H                      # 64, head dim
NCORES = 8
TPG = 4                          # tensor-parallel group size (cores per batch)
HPC = H // TPG                   # 4 heads per core
DSL = HPC * DK                   # 256, d_model slice per core
P = 128                          # partitions
NSLICE = 4                       # sequence slices (pipeline stages / RS chunks)
SLICE = S // NSLICE              # 512
QB = SLICE // P                  # q-blocks of 128 per slice (4)
KT = D // P                      # k-tiles over d_model (8)
MT = DSL // P                    # m-tiles over the 256-dim slice (2)
SB = S // P                      # 16 s-blocks of 128
VW = DK + 2                      # V width per head: [V | ones | ones-pad]
                                 # (fp32r matmul needs even moving size)

F32 = mybir.dt.float32
F32R = mybir.dt.float32r
AF = mybir.ActivationFunctionType
NEG = -1.0e9

_cache = {}

# Set by kernel() when BASSK_TRACE=1 (requires prof_util.install()).
last_exec_time_ns = None
last_profile = None


def _build_program(causal: bool):
    """Build the SPMD Bass program (same program on all 8 cores; per-core
    behavior differs only through input values)."""
    nc = bacc.Bacc("TRN2", target_bir_lowering=False, debug=False,
                   num_devices=NCORES)

    def param(name, shape, dt=F32R):
        return nc.dram_tensor(name, shape, dt, kind="ExternalInput").ap()

    xTq = param("xTq", [D, S])
    xTk = param("xTk", [D, S])
    xTv = param("xTv", [D, S])
    wqT = param("wqT", [D, DSL])
    wkT = param("wkT", [D, DSL])
    wvT = param("wvT", [D, DSL])
    woT = param("woT", [DSL, D])
    bq = param("bq", [MT, P], F32)
    bk = param("bk", [MT, P], F32)
    wo_b = param("wo_b", [D])          # effective wo bias (+ folded v bias)
    mbias = param("mbias", [P, P], F32)  # 0 where k<=q else -1e9 ([k,q] layout)
    ident = param("ident", [P, P])
    ones = param("ones", [P, P])
    if not causal:
        # additive mask in scores^T layout [k, q]: 0 keep / -1e9 drop
        maskT = param("maskT", [S, S], F32)

    out = nc.dram_tensor("out", [SLICE, D], F32, kind="ExternalOutput").ap()

    groups = [[0, 1, 2, 3], [4, 5, 6, 7]]

    def n_kblks(sl):
        return (sl + 1) * QB if causal else SB

    with tile.TileContext(nc) as tc:
        with (
            tc.tile_pool(name="res", bufs=1) as res,
            tc.tile_pool(name="xq", bufs=KT) as xq_pool,
            tc.tile_pool(name="xk", bufs=KT) as xk_pool,
            tc.tile_pool(name="xv", bufs=KT) as xv_pool,
            tc.tile_pool(name="et", bufs=3) as et_pool,
            tc.tile_pool(name="osb", bufs=4) as o_pool,
            tc.tile_pool(name="ysb", bufs=2) as y_pool,
            tc.tile_pool(name="recip", bufs=4) as recip_pool,
            tc.tile_pool(name="mb", bufs=2) as mb_pool,
            # PSUM: 2 (big) + 2 (scores/transpose) + 4 (AV accum) = 8 banks
            tc.tile_pool(name="ps_big", bufs=2, space="PSUM") as ps_big,
            tc.tile_pool(name="ps_sc", bufs=2, space="PSUM") as ps_sc,
            tc.tile_pool(name="ps_av", bufs=4, space="PSUM") as ps_av,
            tc.tile_pool(name="dram", bufs=2 * NSLICE, space="DRAM") as dram_pool,
        ):
            # ---- resident tiles ----
            qt_sb = [res.tile([P, S], F32R, name=f"qt{t}") for t in range(MT)]
            kt_sb = [res.tile([P, S], F32R, name=f"kt{t}") for t in range(MT)]
            # V per s-block: 4 heads x [V_h | 1 | 1] of width 66
            v_sb = [res.tile([P, HPC * VW], F32R, name=f"v{i}") for i in range(SB)]
            for i in range(SB):
                for h in range(HPC):
                    nc.sync.dma_start(
                        v_sb[i][:, h * VW + DK:h * VW + VW], ones[:, 0:2])
            ot_sb = [res.tile([P, S], F32R, name=f"ot{t}") for t in range(MT)]
            wqT_sb = [res.tile([P, DSL], F32R, name=f"wq{k}") for k in range(KT)]
            wkT_sb = [res.tile([P, DSL], F32R, name=f"wk{k}") for k in range(KT)]
            wvT_sb = [res.tile([P, DSL], F32R, name=f"wv{k}") for k in range(KT)]
            woT_sb = [res.tile([P, D], F32R, name=f"wo{k}") for k in range(MT)]
            for k in range(KT):
                nc.sync.dma_start(wqT_sb[k][:], wqT[k * P:(k + 1) * P, :])
                nc.sync.dma_start(wkT_sb[k][:], wkT[k * P:(k + 1) * P, :])
                nc.sync.dma_start(wvT_sb[k][:], wvT[k * P:(k + 1) * P, :])
            for k in range(MT):
                nc.sync.dma_start(woT_sb[k][:], woT[k * P:(k + 1) * P, :])
            bq_sb = res.tile([P, MT], F32)
            bk_sb = res.tile([P, MT], F32)
            nc.sync.dma_start(bq_sb[:], bq.rearrange("m p -> p m"))
            nc.sync.dma_start(bk_sb[:], bk.rearrange("m p -> p m"))
            mbias_sb = res.tile([P, P], F32)
            nc.sync.dma_start(mbias_sb[:], mbias)
            ident_sb = res.tile([P, P], F32R)
            nc.sync.dma_start(ident_sb[:], ident)
            # broadcast wo_b to all 128 partitions via a K=1 matmul
            wo_b_row = res.tile([1, D], F32R)
            nc.sync.dma_start(wo_b_row[:], wo_b[None, :])
            ones_row = res.tile([1, P], F32R)
            nc.sync.dma_start(ones_row[:], ones[0:1, :])
            wo_b_bcast = res.tile([P, D], F32)
            for half in range(2):
                hs = slice(half * (D // 2), (half + 1) * (D // 2))
                pb = ps_big.tile([P, D // 2], F32, name=f"pb{half}", tag="big")
                nc.tensor.matmul(pb[:], ones_row[:], wo_b_row[:, hs],
                                 start=True, stop=True)
                nc.vector.tensor_copy(wo_b_bcast[:, hs], pb[:])

            # ---- main pipeline over sequence slices ----
            def project_slice(sl):
                s0 = sl * SLICE
                # -- projections for this slice --
                xq_t, xk_t, xv_t = [], [], []
                for k in range(KT):
                    xq = xq_pool.tile([P, SLICE], F32R, name=f"xq_{sl}_{k}", tag="x")
                    xk = xk_pool.tile([P, SLICE], F32R, name=f"xk_{sl}_{k}", tag="x")
                    xv = xv_pool.tile([P, SLICE], F32R, name=f"xv_{sl}_{k}", tag="x")
                    nc.sync.dma_start(xq[:], xTq[k * P:(k + 1) * P, s0:s0 + SLICE])
                    nc.sync.dma_start(xk[:], xTk[k * P:(k + 1) * P, s0:s0 + SLICE])
                    nc.sync.dma_start(xv[:], xTv[k * P:(k + 1) * P, s0:s0 + SLICE])
                    xq_t.append(xq)
                    xk_t.append(xk)
                    xv_t.append(xv)
                for m in range(MT):
                    for dst, w_sb, x_t, b_sb in (
                        (qt_sb, wqT_sb, xq_t, bq_sb),
                        (kt_sb, wkT_sb, xk_t, bk_sb),
                    ):
                        pp = ps_big.tile([P, SLICE], F32, name=f"pp_{sl}_{m}",
                                         tag="big")
                        for k in range(KT):
                            nc.tensor.matmul(
                                pp[:],
                                w_sb[k][:, m * P:(m + 1) * P],
                                x_t[k][:],
                                start=(k == 0), stop=(k == KT - 1),
                            )
                        nc.scalar.activation(
                            dst[m][:, s0:s0 + SLICE], pp[:], AF.Identity,
                            bias=b_sb[:, m:m + 1],
                        )
                for qb in range(QB):
                    sb_i = sl * QB + qb
                    pv = ps_big.tile([P, DSL], F32, name=f"pv_{sl}_{qb}",
                                     tag="big")
                    for k in range(KT):
                        nc.tensor.matmul(
                            pv[:],
                            xv_t[k][:, qb * P:(qb + 1) * P],
                            wvT_sb[k][:],
                            start=(k == 0), stop=(k == KT - 1),
                        )
                    for h in range(HPC):
                        nc.vector.tensor_copy(
                            v_sb[sb_i][:, h * VW:h * VW + DK],
                            pv[:, h * DK:(h + 1) * DK],
                        )

            def attend_slice(sl):
                s0 = sl * SLICE
                # -- attention for the 4 q-blocks of this slice --
                for h in range(HPC):
                    t, r0 = h // 2, (h % 2) * DK
                    av_ps = [
                        ps_av.tile([P, VW], F32, name=f"av_{sl}_{h}_{qb}",
                                   tag="av")
                        for qb in range(QB)
                    ]
                    for kb in range(n_kblks(sl)):
                        q_lo = max(kb - sl * QB, 0) if causal else 0
                        nq = SLICE - q_lo * P
                        sc = ps_sc.tile([P, SLICE], F32, name=f"sc_{sl}_{h}",
                                        tag="sc")
                        nc.tensor.matmul(
                            sc[:, :nq],
                            kt_sb[t][r0:r0 + DK, kb * P:(kb + 1) * P],
                            qt_sb[t][r0:r0 + DK, s0 + q_lo * P:s0 + SLICE],
                            start=True, stop=True,
                        )
                        if causal:
                            if kb >= sl * QB:
                                # diagonal block: mask k > q before exp
                                nc.vector.tensor_add(
                                    sc[:, :P], sc[:, :P], mbias_sb[:])
                        else:
                            mb = mb_pool.tile([P, SLICE], F32,
                                              name=f"mb_{sl}_{h}_{kb}", tag="mb")
                            nc.sync.dma_start(
                                mb[:], maskT[kb * P:(kb + 1) * P, s0:s0 + SLICE])
                            nc.vector.tensor_add(sc[:], sc[:], mb[:])
                        et = et_pool.tile([P, SLICE], F32R, name=f"et_{sl}_{h}",
                                          tag="et")
                        nc.scalar.activation(
                            et[:, :nq], sc[:, :nq], AF.Exp,
                            scale=1.0 / float(np.sqrt(DK)),
                        )
                        for qb in range(q_lo, QB):
                            last_kb = sl * QB + qb if causal else SB - 1
                            nc.tensor.matmul(
                                av_ps[qb][:],
                                et[:, (qb - q_lo) * P:(qb - q_lo + 1) * P],
                                v_sb[kb][:, h * VW:(h + 1) * VW],
                                start=(kb == 0), stop=(kb == last_kb),
                            )
                    # normalize + transpose into O^T
                    for qb in range(QB):
                        rec = recip_pool.tile([P, 1], F32, name=f"rc_{sl}_{h}",
                                              tag="rc")
                        nc.vector.reciprocal(rec[:], av_ps[qb][:, DK:DK + 1])
                        o_sb = o_pool.tile([P, DK], F32R, name=f"o_{sl}_{h}",
                                           tag="o")
                        nc.scalar.activation(
                            o_sb[:], av_ps[qb][:, :DK], AF.Copy, scale=rec[:],
                        )
                        ot_ps = ps_sc.tile([DK, P], F32R, name=f"otp_{sl}_{h}",
                                           tag="sc")
                        nc.tensor.transpose(ot_ps[:], o_sb[:], ident_sb[:])
                        nc.vector.tensor_copy(
                            ot_sb[t][r0:r0 + DK,
                                     s0 + qb * P:s0 + (qb + 1) * P],
                            ot_ps[:],
                        )

                # -- output projection for this slice + RS chunk --
                y_dram = dram_pool.tile([SLICE, D], F32, name=f"y_{sl}",
                                        tag="y")
                for qb in range(QB):
                    col = s0 + qb * P
                    y_sb = y_pool.tile([P, D], F32, name=f"y_{sl}_{qb}",
                                       tag="ysb")
                    for half in range(2):
                        hs = slice(half * (D // 2), (half + 1) * (D // 2))
                        po = ps_big.tile([P, D // 2], F32,
                                         name=f"po_{sl}_{qb}", tag="big")
                        for k in range(MT):
                            nc.tensor.matmul(
                                po[:],
                                ot_sb[k][:, col:col + P],
                                woT_sb[k][:, hs],
                                start=(k == 0), stop=(k == MT - 1),
                            )
                        nc.vector.tensor_add(
                            y_sb[:, hs], po[:], wo_b_bcast[:, hs])
                    nc.sync.dma_start(y_dram[qb * P:(qb + 1) * P, :], y_sb[:])
                rs_out = dram_pool.tile([SLICE // TPG, D], F32,
                                        name=f"rs_{sl}", tag="rs")
                nc.gpsimd.collective_compute(
                    "ReduceScatter",
                    mybir.AluOpType.add,
                    replica_groups=groups,
                    ins=[y_dram[:].opt()],
                    outs=[rs_out[:].opt()],
                )
                nc.sync.dma_start(out[sl * P:(sl + 1) * P, :], rs_out[:])

            if causal:
                for sl in range(NSLICE):
                    project_slice(sl)
                    attend_slice(sl)
            else:
                for sl in range(NSLICE):
                    project_slice(sl)
                for sl in range(NSLICE):
                    attend_slice(sl)

    nc.compile()
    return nc


def _get_program(causal: bool):
    if causal not in _cache:
        _cache[causal] = _build_program(causal)
    return _cache[causal]


def _prepare_inputs(q, k, v, mask, wq_w, wq_b, wk_w, wk_b, wv_w, wv_b,
                    wo_w, wo_b, causal):
    kk, qq = np.meshgrid(np.arange(P), np.arange(P), indexing="ij")
    mbias = np.where(kk <= qq, 0.0, NEG).astype(np.float32)
    ident = np.eye(P, dtype=np.float32)
    xT = [[np.ascontiguousarray(x[b].T) for x in (q, k, v)] for b in range(B)]
    per_g = []
    for g in range(TPG):
        hs = slice(g * DSL, (g + 1) * DSL)
        woT = np.ascontiguousarray(wo_w[:, hs].T)
        # fold v bias through attention (softmax rows sum to 1) into wo bias
        wo_b_eff = wv_b[hs].astype(np.float32) @ woT
        if g == 0:
            wo_b_eff = wo_b_eff + wo_b
        per_g.append(dict(
            wqT=np.ascontiguousarray(wq_w[hs, :].T),
            wkT=np.ascontiguousarray(wk_w[hs, :].T),
            wvT=np.ascontiguousarray(wv_w[hs, :].T),
            woT=woT,
            bq=np.ascontiguousarray(wq_b[hs].reshape(MT, P)),
            bk=np.ascontiguousarray(wk_b[hs].reshape(MT, P)),
            wo_b=wo_b_eff.astype(np.float32),
        ))
    in_maps = []
    for c in range(NCORES):
        b, g = divmod(c, TPG)
        m = dict(
            xTq=xT[b][0], xTk=xT[b][1], xTv=xT[b][2],
            mbias=mbias, ident=ident,
            ones=np.ones((P, P), dtype=np.float32), **per_g[g],
        )
        if not causal:
            m["maskT"] = np.ascontiguousarray(
                np.where(mask[0, 0] != 0, 0.0, NEG).astype(np.float32).T)
        in_maps.append(m)
    return in_maps


def _assemble(results):
    full = np.empty((B, S, D), dtype=np.float32)
    for c in range(NCORES):
        b, r = divmod(c, TPG)
        o = results[c]["out"]  # [512, 1024]: chunk i rows -> global i*512+r*128
        for i in range(NSLICE):
            g0 = i * SLICE + r * P
            full[b, g0:g0 + P, :] = o[i * P:(i + 1) * P, :]
    return full


def kernel(**inputs):
    global last_exec_time_ns, last_profile
    mask = np.asarray(inputs["mask"])
    causal = bool(
        np.array_equal(mask[0, 0] != 0,
                       np.tril(np.ones((S, S), dtype=bool))))
    nc = _get_program(causal)
    in_maps = _prepare_inputs(
        np.asarray(inputs["q"], dtype=np.float32),
        np.asarray(inputs["k"], dtype=np.float32),
        np.asarray(inputs["v"], dtype=np.float32),
        mask,
        *(np.asarray(inputs[n], dtype=np.float32) for n in (
            "wq_w", "wq_b", "wk_w", "wk_b", "wv_w", "wv_b", "wo_w", "wo_b")),
        causal=causal,
    )
    trace = os.environ.get("BASSK_TRACE") == "1"
    res = run_bass_kernel_spmd(nc, in_maps, list(range(NCORES)), trace=trace)
    last_exec_time_ns = res.exec_time_ns
    last_profile = res.profile_json
    return _assemble(res.results)


# revision 10
# speedup vs baseline: 1.0376x; 1.0376x over previous
"""Multi-head attention (B=2, S=2048, D=1024, H=16) on 8 Trainium2 NeuronCores.

Sharding: data-parallel on batch (2 ways) x tensor-parallel on heads (4 ways,
4 heads = 256 d_model dims per core), per the problem's sharding hint. Each
core:
  - projects Q^T/K^T (in transposed [256, S] layout) and V ([S, 256+ones])
    for its head slice from host-pre-transposed activations x^T,
  - runs causal attention per head: scores^T tiles -> additive -1e9 mask on
    the diagonal blocks -> exp (no max-subtraction: scores ~ N(0,1) for this
    module's input distribution, verified causal mask on host) -> A@[V|1]
    accumulation where the appended ones column yields the softmax
    denominator for free -> per-row normalization on eviction,
  - PE-transposes the per-head outputs into O^T for the output projection,
  - projects through its Wo column slice (+ folded v/wo biases),
  - ReduceScatters partial outputs (4 chunks of 512 rows, pipelined with
    compute) across the 4 cores of its batch group.
Host reassembles the 8 x [512, 1024] shards into [2, 2048, 1024].

Matmuls run as float32r (full-rate fp32 path on the PE).
"""

import os
import numpy as np

import concourse.bass as bass
import concourse.mybir as mybir
import concourse.tile as tile
from concourse import bacc
from concourse.bass_utils import run_bass_kernel_spmd

B, S, D, H = 2, 2048, 1024, 16
DK = D // H                      # 64, head dim
NCORES = 8
TPG = 4                          # tensor-parallel group size (cores per batch)
HPC = H // TPG                   # 4 heads per core
DSL = HPC * DK                   # 256, d_model slice per core
P = 128                          # partitions
NSLICE = 4                       # sequence slices (pipeline stages / RS chunks)
SLICE = S // NSLICE              # 512
QB = SLICE // P                  # q-blocks of 128 per slice (4)
KT = D // P                      # k-tiles over d_model (8)
MT = DSL // P                    # m-tiles over the 256-dim slice (2)
SB = S // P                      # 16 s-blocks of 128
VW = DK + 2                      # V width per head: [V | ones | ones-pad]
                                 # (fp32r matmul needs even moving size)

F32 = mybir.dt.float32
F32R = mybir.dt.float32r
AF = mybir.ActivationFunctionType
NEG = -1.0e9

_cache = {}

# Set by kernel() when BASSK_TRACE=1 (requires prof_util.install()).
last_exec_time_ns = None
last_profile = None


def _build_program(causal: bool):
    """Build the SPMD Bass program (same program on all 8 cores; per-core
    behavior differs only through input values)."""
    nc = bacc.Bacc("TRN2", target_bir_lowering=False, debug=False,
                   num_devices=NCORES)

    def param(name, shape, dt=F32R):
        return nc.dram_tensor(name, shape, dt, kind="ExternalInput").ap()

    xTq = param("xTq", [D, S])
    xTk = param("xTk", [D, S])
    xTv = param("xTv", [D, S])
    wqT = param("wqT", [D, DSL])
    wkT = param("wkT", [D, DSL])
    wvT = param("wvT", [D, DSL])
    woT = param("woT", [DSL, D])
    bq = param("bq", [MT, P], F32)
    bk = param("bk", [MT, P], F32)
    wo_b = param("wo_b", [D])          # effective wo bias (+ folded v bias)
    mbias = param("mbias", [P, P], F32)  # 0 where k<=q else -1e9 ([k,q] layout)
    ident = param("ident", [P, P])
    ones = param("ones", [P, P])
    if not causal:
        # additive mask in scores^T layout [k, q]: 0 keep / -1e9 drop
        maskT = param("maskT", [S, S], F32)

    out = nc.dram_tensor("out", [SLICE, D], F32, kind="ExternalOutput").ap()

    groups = [[0, 1, 2, 3], [4, 5, 6, 7]]

    def n_kblks(sl):
        return (sl + 1) * QB if causal else SB

    with tile.TileContext(nc) as tc:
        with (
            tc.tile_pool(name="res", bufs=1) as res,
            tc.tile_pool(name="xq", bufs=KT) as xq_pool,
            tc.tile_pool(name="xk", bufs=KT) as xk_pool,
            tc.tile_pool(name="xv", bufs=KT) as xv_pool,
            tc.tile_pool(name="et", bufs=3) as et_pool,
            tc.tile_pool(name="ysb", bufs=2) as y_pool,
            tc.tile_pool(name="rrow", bufs=3) as rrow_pool,
            tc.tile_pool(name="mb", bufs=2) as mb_pool,
            # PSUM: 2 (big) + 2 (scores) + 4 (AV accum / denom bcast) = 8
            tc.tile_pool(name="ps_big", bufs=2, space="PSUM") as ps_big,
            tc.tile_pool(name="ps_sc", bufs=2, space="PSUM") as ps_sc,
            tc.tile_pool(name="ps_av", bufs=4, space="PSUM") as ps_av,
            tc.tile_pool(name="dram", bufs=2 * NSLICE, space="DRAM") as dram_pool,
        ):
            # ---- resident tiles ----
            qt_sb = [res.tile([P, S], F32R, name=f"qt{t}") for t in range(MT)]
            kt_sb = [res.tile([P, S], F32R, name=f"kt{t}") for t in range(MT)]
            # V per s-block: 4 heads x [V_h | 1 | 1] of width 66
            v_sb = [res.tile([P, HPC * VW], F32R, name=f"v{i}") for i in range(SB)]
            for i in range(SB):
                for h in range(HPC):
                    nc.sync.dma_start(
                        v_sb[i][:, h * VW + DK:h * VW + VW], ones[:, 0:2])
            ot_sb = [res.tile([P, S], F32R, name=f"ot{t}") for t in range(MT)]
            wqT_sb = [res.tile([P, DSL], F32R, name=f"wq{k}") for k in range(KT)]
            wkT_sb = [res.tile([P, DSL], F32R, name=f"wk{k}") for k in range(KT)]
            wvT_sb = [res.tile([P, DSL], F32R, name=f"wv{k}") for k in range(KT)]
            woT_sb = [res.tile([P, D], F32R, name=f"wo{k}") for k in range(MT)]
            for k in range(KT):
                nc.sync.dma_start(wqT_sb[k][:], wqT[k * P:(k + 1) * P, :])
                nc.sync.dma_start(wkT_sb[k][:], wkT[k * P:(k + 1) * P, :])
                nc.sync.dma_start(wvT_sb[k][:], wvT[k * P:(k + 1) * P, :])
            for k in range(MT):
                nc.sync.dma_start(woT_sb[k][:], woT[k * P:(k + 1) * P, :])
            bq_sb = res.tile([P, MT], F32)
            bk_sb = res.tile([P, MT], F32)
            nc.sync.dma_start(bq_sb[:], bq.rearrange("m p -> p m"))
            nc.sync.dma_start(bk_sb[:], bk.rearrange("m p -> p m"))
            mbias_sb = res.tile([P, P], F32)
            nc.sync.dma_start(mbias_sb[:], mbias)
            ident_sb = res.tile([P, P], F32R)
            nc.sync.dma_start(ident_sb[:], ident)
            # broadcast wo_b to all 128 partitions via a K=1 matmul
            wo_b_row = res.tile([1, D], F32R)
            nc.sync.dma_start(wo_b_row[:], wo_b[None, :])
            ones_row = res.tile([1, P], F32R)
            nc.sync.dma_start(ones_row[:], ones[0:1, :])
            wo_b_bcast = res.tile([P, D], F32)
            for half in range(2):
                hs = slice(half * (D // 2), (half + 1) * (D // 2))
                pb = ps_big.tile([P, D // 2], F32, name=f"pb{half}", tag="big")
                nc.tensor.matmul(pb[:], ones_row[:], wo_b_row[:, hs],
                                 start=True, stop=True)
                nc.vector.tensor_copy(wo_b_bcast[:, hs], pb[:])

            # ---- main pipeline over sequence slices ----
            def project_slice(sl):
                s0 = sl * SLICE
                # -- projections for this slice --
                xq_t, xk_t, xv_t = [], [], []
                for k in range(KT):
                    xq = xq_pool.tile([P, SLICE], F32R, name=f"xq_{sl}_{k}", tag="x")
                    xk = xk_pool.tile([P, SLICE], F32R, name=f"xk_{sl}_{k}", tag="x")
                    xv = xv_pool.tile([P, SLICE], F32R, name=f"xv_{sl}_{k}", tag="x")
                    nc.sync.dma_start(xq[:], xTq[k * P:(k + 1) * P, s0:s0 + SLICE])
                    nc.sync.dma_start(xk[:], xTk[k * P:(k + 1) * P, s0:s0 + SLICE])
                    nc.sync.dma_start(xv[:], xTv[k * P:(k + 1) * P, s0:s0 + SLICE])
                    xq_t.append(xq)
                    xk_t.append(xk)
                    xv_t.append(xv)
                for m in range(MT):
                    for dst, w_sb, x_t, b_sb in (
                        (qt_sb, wqT_sb, xq_t, bq_sb),
                        (kt_sb, wkT_sb, xk_t, bk_sb),
                    ):
                        pp = ps_big.tile([P, SLICE], F32, name=f"pp_{sl}_{m}",
                                         tag="big")
                        for k in range(KT):
                            nc.tensor.matmul(
                                pp[:],
                                w_sb[k][:, m * P:(m + 1) * P],
                                x_t[k][:],
                                start=(k == 0), stop=(k == KT - 1),
                            )
                        nc.scalar.activation(
                            dst[m][:, s0:s0 + SLICE], pp[:], AF.Identity,
                            bias=b_sb[:, m:m + 1],
                        )
                for qb in range(QB):
                    sb_i = sl * QB + qb
                    pv = ps_big.tile([P, DSL], F32, name=f"pv_{sl}_{qb}",
                                     tag="big")
                    for k in range(KT):
                        nc.tensor.matmul(
                            pv[:],
                            xv_t[k][:, qb * P:(qb + 1) * P],
                            wvT_sb[k][:],
                            start=(k == 0), stop=(k == KT - 1),
                        )
                    for h in range(HPC):
                        nc.vector.tensor_copy(
                            v_sb[sb_i][:, h * VW:h * VW + DK],
                            pv[:, h * DK:(h + 1) * DK],
                        )

            def attend_slice(sl):
                s0 = sl * SLICE
                # -- attention for this slice (O^T-form A@V, N=512) --
                last_kb = n_kblks(sl) - 1
                for h in range(HPC):
                    t, r0 = h // 2, (h % 2) * DK
                    # rows 0-63: O^T accum; row 64: softmax denom; 65: pad
                    av = ps_av.tile([VW, SLICE], F32, name=f"av_{sl}_{h}",
                                    tag="av")
                    for kb in range(n_kblks(sl)):
                        q_lo = max(kb - sl * QB, 0) if causal else 0
                        nq = SLICE - q_lo * P
                        sc = ps_sc.tile([P, SLICE], F32, name=f"sc_{sl}_{h}",
                                        tag="sc")
                        nc.tensor.matmul(
                            sc[:, :nq],
                            kt_sb[t][r0:r0 + DK, kb * P:(kb + 1) * P],
                            qt_sb[t][r0:r0 + DK, s0 + q_lo * P:s0 + SLICE],
                            start=True, stop=True,
                        )
                        if causal:
                            if kb >= sl * QB:
                                # diagonal block: mask k > q before exp
                                nc.vector.tensor_add(
                                    sc[:, :P], sc[:, :P], mbias_sb[:])
                        else:
                            mb = mb_pool.tile([P, SLICE], F32,
                                              name=f"mb_{sl}_{h}_{kb}", tag="mb")
                            nc.sync.dma_start(
                                mb[:], maskT[kb * P:(kb + 1) * P, s0:s0 + SLICE])
                            nc.vector.tensor_add(sc[:], sc[:], mb[:])
                        et = et_pool.tile([P, SLICE], F32R, name=f"et_{sl}_{h}",
                                          tag="et")
                        nc.scalar.activation(
                            et[:, :nq], sc[:, :nq], AF.Exp,
                            scale=1.0 / float(np.sqrt(DK)),
                        )
                        # accumulate [V|1].T @ exp(S^T) into O^T + denom row.
                        # band kbs only touch columns >= q_lo*P; every column
                        # got its start write at kb=0, so flags stay simple.
                        nc.tensor.matmul(
                            av[:, q_lo * P:SLICE],
                            v_sb[kb][:, h * VW:(h + 1) * VW],
                            et[:, :nq],
                            start=(kb == 0), stop=(kb == last_kb),
                            skip_group_check=(kb != 0 and kb != last_kb),
                        )
                    # normalize: recip of denom row, PE-broadcast to 64 rows,
                    # multiply straight into resident O^T
                    rrow = rrow_pool.tile([1, SLICE], F32R,
                                          name=f"rr_{sl}_{h}", tag="rr")
                    with nc.allow_low_precision(
                            reason="f32r is 4-byte fp32 bits for the PE"):
                        nc.vector.reciprocal(rrow[:], av[DK:DK + 1, :])
                    bc = ps_av.tile([DK, SLICE], F32, name=f"bc_{sl}_{h}",
                                    tag="av")
                    nc.tensor.matmul(bc[:], ones_row[0:1, 0:DK], rrow[:],
                                     start=True, stop=True)
                    bc_sb = rrow_pool.tile([DK, SLICE], F32,
                                           name=f"bcs_{sl}_{h}", tag="bcs")
                    nc.vector.tensor_copy(bc_sb[:], bc[:])
                    nc.vector.tensor_mul(
                        ot_sb[t][r0:r0 + DK, s0:s0 + SLICE],
                        av[0:DK, :], bc_sb[:],
                    )

                # -- output projection for this slice + RS chunk --
                y_dram = dram_pool.tile([SLICE, D], F32, name=f"y_{sl}",
                                        tag="y")
                for qb in range(QB):
                    col = s0 + qb * P
                    y_sb = y_pool.tile([P, D], F32, name=f"y_{sl}_{qb}",
                                       tag="ysb")
                    for half in range(2):
                        hs = slice(half * (D // 2), (half + 1) * (D // 2))
                        po = ps_big.tile([P, D // 2], F32,
                                         name=f"po_{sl}_{qb}", tag="big")
                        for k in range(MT):
                            nc.tensor.matmul(
                                po[:],
                                ot_sb[k][:, col:col + P],
                                woT_sb[k][:, hs],
                                start=(k == 0), stop=(k == MT - 1),
                            )
                        nc.vector.tensor_add(
                            y_sb[:, hs], po[:], wo_b_bcast[:, hs])
                    nc.sync.dma_start(y_dram[qb * P:(qb + 1) * P, :], y_sb[:])
                rs_out = dram_pool.tile([SLICE // TPG, D], F32,
                                        name=f"rs_{sl}", tag="rs")
                nc.gpsimd.collective_compute(
                    "ReduceScatter",
                    mybir.AluOpType.add,
                    replica_groups=groups,
                    ins=[y_dram[:].opt()],
                    outs=[rs_out[:].opt()],
                )
                nc.sync.dma_start(out[sl * P:(sl + 1) * P, :], rs_out[:])

            if causal:
                for sl in range(NSLICE):
                    project_slice(sl)
                    attend_slice(sl)
            else:
                for sl in range(NSLICE):
                    project_slice(sl)
                for sl in range(NSLICE):
                    attend_slice(sl)

    nc.compile()
    return nc


def _get_program(causal: bool):
    if causal not in _cache:
        _cache[causal] = _build_program(causal)
    return _cache[causal]


def _prepare_inputs(q, k, v, mask, wq_w, wq_b, wk_w, wk_b, wv_w, wv_b,
                    wo_w, wo_b, causal):
    kk, qq = np.meshgrid(np.arange(P), np.arange(P), indexing="ij")
    mbias = np.where(kk <= qq, 0.0, NEG).astype(np.float32)
    ident = np.eye(P, dtype=np.float32)
    xT = [[np.ascontiguousarray(x[b].T) for x in (q, k, v)] for b in range(B)]
    per_g = []
    for g in range(TPG):
        hs = slice(g * DSL, (g + 1) * DSL)
        woT = np.ascontiguousarray(wo_w[:, hs].T)
        # fold v bias through attention (softmax rows sum to 1) into wo bias
        wo_b_eff = wv_b[hs].astype(np.float32) @ woT
        if g == 0:
            wo_b_eff = wo_b_eff + wo_b
        per_g.append(dict(
            wqT=np.ascontiguousarray(wq_w[hs, :].T),
            wkT=np.ascontiguousarray(wk_w[hs, :].T),
            wvT=np.ascontiguousarray(wv_w[hs, :].T),
            woT=woT,
            bq=np.ascontiguousarray(wq_b[hs].reshape(MT, P)),
            bk=np.ascontiguousarray(wk_b[hs].reshape(MT, P)),
            wo_b=wo_b_eff.astype(np.float32),
        ))
    in_maps = []
    for c in range(NCORES):
        b, g = divmod(c, TPG)
        m = dict(
            xTq=xT[b][0], xTk=xT[b][1], xTv=xT[b][2],
            mbias=mbias, ident=ident,
            ones=np.ones((P, P), dtype=np.float32), **per_g[g],
        )
        if not causal:
            m["maskT"] = np.ascontiguousarray(
                np.where(mask[0, 0] != 0, 0.0, NEG).astype(np.float32).T)
        in_maps.append(m)
    return in_maps


def _assemble(results):
    full = np.empty((B, S, D), dtype=np.float32)
    for c in range(NCORES):
        b, r = divmod(c, TPG)
        o = results[c]["out"]  # [512, 1024]: chunk i rows -> global i*512+r*128
        for i in range(NSLICE):
            g0 = i * SLICE + r * P
            full[b, g0:g0 + P, :] = o[i * P:(i + 1) * P, :]
    return full


def kernel(**inputs):
    global last_exec_time_ns, last_profile
    mask = np.asarray(inputs["mask"])
    causal = bool(
        np.array_equal(mask[0, 0] != 0,
                       np.tril(np.ones((S, S), dtype=bool))))
    nc = _get_program(causal)
    in_maps = _prepare_inputs(
        np.asarray(inputs["q"], dtype=np.float32),
        np.asarray(inputs["k"], dtype=np.float32),
        np.asarray(inputs["v"], dtype=np.float32),
        mask,
        *(np.asarray(inputs[n], dtype=np.float32) for n in (
            "wq_w", "wq_b", "wk_w", "wk_b", "wv_w", "wv_b", "wo_w", "wo_b")),
        causal=causal,
    )
    trace = os.environ.get("BASSK_TRACE") == "1"
    res = run_bass_kernel_spmd(nc, in_maps, list(range(NCORES)), trace=trace)
    last_exec_time_ns = res.exec_time_ns
    last_profile = res.profile_json
    return _assemble(res.results)
